# revision 17
# baseline (speedup 1.0000x reference)
"""Trainium2 Bass kernel for a GraphTransformer message-passing layer.

Contract: kernel(**inputs) takes the FULL unsharded inputs (numpy arrays, keyed
as in setup_inputs()) and returns the FULL [N, D] float32 output.

Strategy (8 NeuronCores, SPMD):
  * Edges are sorted by dst and sharded so core c owns the contiguous node
    range [c*N/8, (c+1)*N/8) plus every edge pointing into it.
  * The host gathers src/dst raw node features per edge (transposed, bf16) so
    each core computes K/V/Q projections per edge with per-tile-stationary
    matmuls (features on partitions -> edges on PSUM partitions).
  * Per 128-node window, a one-hot scatter matrix M (built on-chip from the
    window-relative dst index via iota+is_equal) turns segment-sum into PSUM
    matmul accumulation.  z rides along as 4 extra columns.
  * BatchNorm statistics are computed on-device (partial sums per core, tiny
    AllReduce across the 8 cores); the BN scale is folded into the projection
    weights, BN bias rows are added into PSUM with rank-1 (ones) matmuls.
  * Node-side GEMMs (O/W1/W2) run feature-major in fp32r, data-parallel over
    the node dim, with a second tiny AllReduce for the mid-layer BN.
"""

from contextlib import ExitStack

import numpy as np
import ml_dtypes

import concourse.bass as bass
import concourse.bacc as bacc
import concourse.mybir as mybir
import concourse.tile as tile
from concourse.masks import make_identity

BF16 = ml_dtypes.bfloat16
F32 = mybir.dt.float32
F32R = mybir.dt.float32r
BF = mybir.dt.bfloat16
I32 = mybir.dt.int32
AX = mybir.AxisListType
OP = mybir.AluOpType
AF = mybir.ActivationFunctionType

P = 128


def default_cfg():
    return dict(N=50000, E=800000, D=128, H=4, NCORES=8)


# ----------------------------------------------------------------------------
# Host-side preprocessing
# ----------------------------------------------------------------------------

def host_prep(inputs, cfg):
    """Sort/shard/pad edges, gather+transpose features, build aux arrays.

    Returns (in_maps, meta). meta carries the compile-time loop structure.
    """
    N, E, D, NC = cfg["N"], cfg["E"], cfg["D"], cfg["NCORES"]
    NPC = N // NC                      # nodes per core
    NWIN = (NPC + P - 1) // P          # 128-node windows per core
    NPAD = NWIN * P

    nf = np.ascontiguousarray(inputs["node_feats"], dtype=np.float32)
    ef = np.ascontiguousarray(inputs["edge_feats"], dtype=np.float32)
    src = np.asarray(inputs["src"], dtype=np.int64)
    dst = np.asarray(inputs["dst"], dtype=np.int64)

    order = np.argsort(dst, kind="stable")
    dsts = dst[order]

    # Tiles per (core, window), equalized over cores so the SPMD program is
    # identical on every core (scatter-window start/stop flags are static).
    counts = np.zeros((NC, NWIN), dtype=np.int64)
    bounds = np.searchsorted(dsts, np.arange(NC + 1) * NPC)
    for c in range(NC):
        dl = dsts[bounds[c]:bounds[c + 1]] - c * NPC
        wc = np.bincount(dl // P, minlength=NWIN)
        counts[c] = wc[:NWIN]
    Tw = np.maximum(1, -(-counts // P)).max(axis=0)     # [NWIN] ceil-div, >=1
    # pad total tile count to a multiple of the chunk size (4 tiles)
    extra = (-int(Tw.sum())) % 4
    Tw[-1] += extra
    Woff = np.concatenate([[0], np.cumsum(Tw)])
    TC = int(Tw.sum())                 # tiles per core
    EPC = TC * P                       # padded edges per core

    nfb = nf.astype(BF16)
    efb = ef.astype(BF16)

    in_maps = []
    for c in range(NC):
        eidx = np.full(EPC, -1, dtype=np.int64)          # global edge id, -1 = pad
        dwin = np.full(EPC, -1.0, dtype=np.float32)      # window-relative dst
        seg = order[bounds[c]:bounds[c + 1]]
        dl = dsts[bounds[c]:bounds[c + 1]] - c * NPC
        wstart = np.searchsorted(dl // P, np.arange(NWIN + 1))
        for w in range(NWIN):
            ids = seg[wstart[w]:wstart[w + 1]]
            o = int(Woff[w]) * P
            eidx[o:o + len(ids)] = ids
            dwin[o:o + len(ids)] = (dl[wstart[w]:wstart[w + 1]] - w * P).astype(np.float32)

        real = eidx >= 0
        e_safe = np.where(real, eidx, 0)
        s_idx = np.where(real, src[e_safe], 0)
        d_idx = np.where(real, dst[e_safe], 0)
        gs = nfb[s_idx]; gs[~real] = 0
        gd = nfb[d_idx]; gd[~real] = 0
        ge = efb[e_safe]; ge[~real] = 0

        nfT = np.zeros((P, NPAD), dtype=np.float32)
        nfT[:, :NPC] = nf[c * NPC:(c + 1) * NPC].T

        m = {
            "gsT": np.ascontiguousarray(gs.T),
            "gdT": np.ascontiguousarray(gd.T),
            "efT": np.ascontiguousarray(ge.T),
            "dstw": np.ascontiguousarray(dwin.reshape(TC, P).T),  # [128, TC] f32
            "nfT": nfT,
            "Wq": inputs["Wq"].astype(np.float32),
            "Wk": inputs["Wk"].astype(np.float32),
            "Wv": inputs["Wv"].astype(np.float32),
            "We": inputs["We"].astype(np.float32),
            "O_w": inputs["O_w"].astype(np.float32),
            "W1": inputs["W1"].astype(np.float32),
            "W2": inputs["W2"].astype(np.float32),
            "O_b": np.asarray(inputs["O_b"], np.float32).reshape(P, 1),
            "g1n": np.asarray(inputs["g1n"], np.float32).reshape(P, 1),
            "b1n": np.asarray(inputs["b1n"], np.float32).reshape(P, 1),
            "g1e": np.asarray(inputs["g1e"], np.float32).reshape(P, 1),
            "b1e": np.asarray(inputs["b1e"], np.float32).reshape(P, 1),
            "g2": np.asarray(inputs["g2"], np.float32).reshape(P, 1),
            "b2": np.asarray(inputs["b2"], np.float32).reshape(P, 1),
        }
        in_maps.append(m)

    meta = dict(NPC=NPC, NWIN=NWIN, NPAD=NPAD, TC=TC, EPC=EPC,
                Tw=[int(x) for x in Tw], Woff=[int(x) for x in Woff])
    return in_maps, meta


# ----------------------------------------------------------------------------
# Device program
# ----------------------------------------------------------------------------

def build_graph(ctx: ExitStack, tc: tile.TileContext, io: dict, cfg, meta):
    nc = tc.nc
    N, E, D, H, NC = cfg["N"], cfg["E"], cfg["D"], cfg["H"], cfg["NCORES"]
    DH = D // H
    NPC, NWIN, NPAD, TC, EPC = (meta[k] for k in ("NPC", "NWIN", "NPAD", "TC", "EPC"))
    Tw, Woff = meta["Tw"], meta["Woff"]
    CH = 4                    # tiles per chunk
    CHE = CH * P              # edges per chunk (512)
    NCHUNK = TC // CH
    DH_SCALE = float(1.0 / np.sqrt(np.float32(DH)))
    EXP5 = float(np.exp(np.float32(5.0)))
    EXPM5 = float(np.exp(np.float32(-5.0)))

    cc_group = [list(range(NC))]

    # ---- resident tensors -------------------------------------------------
    res = ctx.enter_context(tc.tile_pool(name="resident", bufs=1))
    dram = ctx.enter_context(tc.tile_pool(name="dram", bufs=1, space="DRAM"))

    ident = res.tile([P, P], F32)
    make_identity(nc, ident[:])

    iota_i = res.tile([P, P], I32)
    nc.gpsimd.iota(iota_i[:], pattern=[[1, P]], base=0, channel_multiplier=0)
    iota_f = res.tile([P, P], F32)
    nc.vector.tensor_copy(iota_f[:], iota_i[:])

    nfT = res.tile([P, NPAD], F32)
    nc.sync.dma_start(nfT[:], io["nfT"][:])
    dstw = res.tile([P, TC], F32)
    nc.sync.dma_start(dstw[:], io["dstw"][:])

    w_sb = {}
    for w in ("Wq", "Wk", "Wv", "We", "O_w", "W1"):
        t = res.tile([P, io[w].shape[1]], F32, tag=f"w_{w}")
        nc.sync.dma_start(t[:], io[w][:])
        w_sb[w] = t
    w2 = res.tile([P, 2 * P], F32, tag="w_W2")   # W2 [256,128] as two halves
    nc.sync.dma_start(w2[:, :P], io["W2"][:P, :])
    nc.sync.dma_start(w2[:, P:], io["W2"][P:, :])
    w_sb["W2"] = w2

    vec_sb = {}
    for v in ("O_b", "g1n", "b1n", "g1e", "b1e", "g2", "b2"):
        t = res.tile([P, 1], F32, tag=f"v_{v}")
        nc.sync.dma_start(t[:], io[v][:])
        vec_sb[v] = t

    ones_row = res.tile([1, P], BF)
    nc.vector.memset(ones_row[:], 1.0)

    Ow_b = res.tile([P, P], BF)
    nc.vector.tensor_copy(Ow_b[:], w_sb["O_w"][:])
    W1_b = res.tile([P, 2 * P], BF)
    nc.vector.tensor_copy(W1_b[:], w_sb["W1"][:])
    W2_b = res.tile([P, 2 * P], BF)
    nc.vector.tensor_copy(W2_b[:], w_sb["W2"][:])

    # ---- phase 1: BN statistics ------------------------------------------
    stat = ctx.enter_context(tc.tile_pool(name="stat", bufs=1))
    junk = stat.tile([P, CHE], F32)

    def combine6(st6, n, tag):
        """Combine n bn_stats outputs [P, n*6] -> (sum_x, sum_x2) [P,1]."""
        v = st6[:].rearrange("p (n t s) -> p n t s", t=2, s=3)
        cnt, mean, m2 = v[:, :, :, 0:1], v[:, :, :, 1:2], v[:, :, :, 2:3]
        cm = stat.tile([P, n * 2], F32, tag=f"cm{tag}", name=f"cm{tag}")
        cm4 = cm[:].rearrange("p (n t s) -> p n t s", t=2, s=1)
        nc.vector.tensor_tensor(out=cm4, in0=cnt, in1=mean, op=OP.mult)
        sx = stat.tile([P, 1], F32, tag=f"sx{tag}", name=f"sx{tag}")
        nc.vector.reduce_sum(out=sx[:], in_=cm[:], axis=AX.X)
        cmm = stat.tile([P, n * 2], F32, tag=f"cmm{tag}", name=f"cmm{tag}")
        cmm4 = cmm[:].rearrange("p (n t s) -> p n t s", t=2, s=1)
        nc.vector.tensor_tensor(out=cmm4, in0=cm4, in1=mean, op=OP.mult)
        sxx = stat.tile([P, 1], F32, tag=f"sxx{tag}", name=f"sxx{tag}")
        nc.vector.reduce_sum(out=sxx[:], in_=cmm[:], axis=AX.X)
        m2s = stat.tile([P, 1], F32, tag=f"m2s{tag}", name=f"m2s{tag}")
        nc.vector.tensor_reduce(out=m2s[:], in_=m2, axis=AX.XYZ, op=OP.add)
        nc.vector.tensor_tensor(out=sxx[:], in0=sxx[:], in1=m2s[:], op=OP.add)
        return sx, sxx

    nch_n = (NPAD + CHE - 1) // CHE
    nst6 = stat.tile([P, nch_n * 6], F32)
    for i in range(nch_n):
        sl = nfT[:, i * CHE:min((i + 1) * CHE, NPAD)]
        nc.vector.bn_stats(nst6[:, i * 6:(i + 1) * 6], sl)
    nsx, nsxx = combine6(nst6, nch_n, "n")

    nch_e = EPC // CHE
    n6 = (nch_e + 1) // 2
    nalt = nch_e - n6
    est6 = stat.tile([P, n6 * 6], F32)
    esum_p = stat.tile([P, nalt], F32)
    esq_p = stat.tile([P, nalt], F32)
    with tc.tile_pool(name="sload", bufs=4) as sload:
        for i in range(nch_e):
            ch = sload.tile([P, CHE], BF, tag="statload")
            nc.sync.dma_start(ch[:], io["efT"][:, i * CHE:(i + 1) * CHE])
            if i % 2 == 0:
                k = i // 2
                nc.vector.bn_stats(est6[:, k * 6:(k + 1) * 6], ch[:])
            else:
                k = i // 2
                nc.scalar.activation(junk[:], ch[:], AF.Square,
                                     accum_out=esq_p[:, k:k + 1])
                nc.vector.reduce_sum(out=esum_p[:, k:k + 1], in_=ch[:], axis=AX.X)
    esx, esxx = combine6(est6, n6, "e")
    if nalt:
        ex2 = stat.tile([P, 1], F32)
        nc.vector.reduce_sum(out=ex2[:], in_=esum_p[:], axis=AX.X)
        nc.vector.tensor_tensor(out=esx[:], in0=esx[:], in1=ex2[:], op=OP.add)
        nc.vector.reduce_sum(out=ex2[:], in_=esq_p[:], axis=AX.X)
        nc.vector.tensor_tensor(out=esxx[:], in0=esxx[:], in1=ex2[:], op=OP.add)

    stats4 = stat.tile([P, 4], F32)
    nc.vector.tensor_copy(stats4[:, 0:1], nsx[:])
    nc.vector.tensor_copy(stats4[:, 1:2], nsxx[:])
    nc.vector.tensor_copy(stats4[:, 2:3], esx[:])
    nc.vector.tensor_copy(stats4[:, 3:4], esxx[:])

    cc1_in = dram.tile([P, 4], F32)
    cc1_out = dram.tile([P, 4], F32)
    nc.sync.dma_start(cc1_in[:], stats4[:])
    nc.gpsimd.collective_compute(
        "AllReduce", OP.add, replica_groups=cc_group,
        ins=[cc1_in.opt()], outs=[cc1_out.opt()])
    gstats = stat.tile([P, 4], F32)
    nc.sync.dma_start(gstats[:], cc1_out[:])

    # ---- phase 2: fold BN into weights -----------------------------------
    fold = ctx.enter_context(tc.tile_pool(name="fold", bufs=1))

    def rsqrt_newton(dst_ap, var_ap, tag):
        """dst = 1/sqrt(var + eps), one Newton refinement on top of sqrt+recip."""
        veps = fold.tile([P, 1], F32, tag=f"veps{tag}")
        nc.vector.tensor_scalar_add(veps[:], var_ap, 1e-5)
        s0 = fold.tile([P, 1], F32, tag=f"s0{tag}")
        nc.scalar.activation(s0[:], veps[:], AF.Sqrt)
        r0 = fold.tile([P, 1], F32, tag=f"r0{tag}")
        nc.vector.reciprocal(r0[:], s0[:])
        t1 = fold.tile([P, 1], F32, tag=f"t1{tag}")
        nc.vector.tensor_tensor(out=t1[:], in0=r0[:], in1=r0[:], op=OP.mult)
        nc.vector.tensor_tensor(out=t1[:], in0=t1[:], in1=veps[:], op=OP.mult)
        nc.vector.tensor_scalar(out=t1[:], in0=t1[:], scalar1=-0.5, scalar2=1.5,
                                op0=OP.mult, op1=OP.add)
        nc.vector.tensor_tensor(out=dst_ap, in0=r0[:], in1=t1[:], op=OP.mult)

    def bn_coeffs(sum_ap, sq_ap, count, g_ap, b_ap, tag):
        mu = fold.tile([P, 1], F32, tag=f"mu{tag}")
        var = fold.tile([P, 1], F32, tag=f"var{tag}")
        sq = fold.tile([P, 1], F32, tag=f"sq{tag}")
        nc.vector.tensor_scalar_mul(mu[:], sum_ap, 1.0 / count)
        nc.vector.tensor_scalar_mul(var[:], sq_ap, 1.0 / count)
        nc.vector.tensor_tensor(out=sq[:], in0=mu[:], in1=mu[:], op=OP.mult)
        nc.vector.tensor_tensor(out=var[:], in0=var[:], in1=sq[:], op=OP.subtract)
        rs = fold.tile([P, 1], F32, tag=f"rs{tag}")
        rsqrt_newton(rs[:], var[:], tag)
        scl = fold.tile([P, 1], F32, tag=f"scl{tag}")
        cb = fold.tile([P, 1], F32, tag=f"cb{tag}")
        nc.vector.tensor_tensor(out=scl[:], in0=rs[:], in1=g_ap, op=OP.mult)
        nc.vector.tensor_tensor(out=cb[:], in0=mu[:], in1=scl[:], op=OP.mult)
        nc.vector.tensor_tensor(out=cb[:], in0=b_ap, in1=cb[:], op=OP.subtract)
        return scl, cb

    scl_n, cb_n = bn_coeffs(gstats[:, 0:1], gstats[:, 1:2], N,
                            vec_sb["g1n"][:], vec_sb["b1n"][:], "n")
    scl_e, cb_e = bn_coeffs(gstats[:, 2:3], gstats[:, 3:4], E,
                            vec_sb["g1e"][:], vec_sb["b1e"][:], "e")

    # scaled weights, bf16
    wf = {}
    for nm, w, scl in (("Wq", "Wq", scl_n), ("Wk", "Wk", scl_n),
                       ("Wv", "Wv", scl_n), ("We", "We", scl_e)):
        t = res.tile([P, P], BF, tag=f"bf_{nm}")
        nc.vector.tensor_scalar_mul(t[:], w_sb[w][:], scl[:, :1])
        wf[nm] = t

    # bias rows: (cb @ W_raw) replicated x4 into [1, 4*128] bf16 rows
    brow = {}
    bv_rep = res.tile([P, P], F32)
    with tc.tile_pool(name="bias_ps", bufs=2, space="PSUM") as bias_ps:
        for nm, raw, cb in (("bq", "Wq", cb_n), ("bk", "Wk", cb_n),
                            ("bv", "Wv", cb_n), ("be", "We", cb_e)):
            raw_b = fold.tile([P, P], BF, tag=f"raw_{nm}")
            nc.vector.tensor_copy(raw_b[:], w_sb[raw][:])
            cb16 = fold.tile([P, 1], BF, tag=f"cb16_{nm}")
            nc.vector.tensor_copy(cb16[:], cb[:])
            ps = bias_ps.tile([1, P], F32, tag="bps")
            nc.tensor.matmul(ps[:], lhsT=cb16[:], rhs=raw_b[:], start=True, stop=True)
            row4 = res.tile([1, 4 * P], BF, tag=f"row4_{nm}")
            for r in range(4):
                nc.vector.tensor_copy(row4[:, r * P:(r + 1) * P], ps[:])
            brow[nm] = row4
        # bv replicated over partitions as [128, 128] fp32 (for z-fold at evac)
        bv_ps = bias_ps.tile([P, P], F32, tag="bvrep")
        nc.tensor.matmul(bv_ps[:], lhsT=ones_row[:], rhs=brow["bv"][:, :P],
                         start=True, stop=True)
        nc.vector.tensor_copy(bv_rep[:], bv_ps[:])

    with tc.tile_critical():
        nc.all_engine_barrier()

    # ---- phase 3: main edge loop -----------------------------------------
    hT = res.tile([P, NPAD], BF)

    t2w = []
    for w in range(NWIN):
        t2w += [w] * Tw[w]

    with (
        tc.tile_pool(name="loads", bufs=3) as lp,
        tc.tile_pool(name="mids", bufs=2) as mp,
        tc.tile_pool(name="mm_ps", bufs=1, space="PSUM") as pp,
        tc.tile_pool(name="scat_ps", bufs=2, space="PSUM") as sp,
        tc.tile_pool(name="evac", bufs=2) as ep,
    ):
        scat_tiles = {}

        def window_evac(w, scat):
            zsb = ep.tile([P, H], F32, tag="zsb")
            nc.vector.tensor_copy(zsb[:], scat[:, D:D + H])
            zeps = ep.tile([P, H], F32, tag="zeps")
            nc.vector.tensor_scalar_add(zeps[:], zsb[:], 1e-6)
            rec = ep.tile([P, H], F32, tag="rec")
            nc.vector.reciprocal(rec[:], zeps[:])
            wvb = ep.tile([P, D], F32, tag="wvb")
            nc.vector.tensor_tensor(
                out=wvb[:].rearrange("p (h d) -> p h d", d=DH),
                in0=bv_rep[:].rearrange("p (h d) -> p h d", d=DH),
                in1=zsb[:].to_broadcast([P, H, DH]), op=OP.mult)
            nc.vector.tensor_tensor(out=wvb[:], in0=wvb[:], in1=scat[:, :D], op=OP.add)
            hdiv = ep.tile([P, D], F32, tag="hdiv")
            nc.vector.tensor_tensor(
                out=hdiv[:].rearrange("p (h d) -> p h d", d=DH),
                in0=wvb[:].rearrange("p (h d) -> p h d", d=DH),
                in1=rec[:].to_broadcast([P, H, DH]), op=OP.mult)
            tps = sp.tile([P, P], F32, tag="tr_ps", bufs=1)
            nc.tensor.transpose(out=tps[:], in_=hdiv[:], identity=ident[:])
            nc.scalar.copy(hT[:, w * P:(w + 1) * P], tps[:])

        for c in range(NCHUNK):
            t0 = c * CH

            gs = lp.tile([P, CHE], BF, tag="gs")
            gd = lp.tile([P, CHE], BF, tag="gd")
            ge = lp.tile([P, CHE], BF, tag="ge")
            nc.sync.dma_start(gs[:], io["gsT"][:, t0 * P:(t0 + CH) * P])
            nc.sync.dma_start(gd[:], io["gdT"][:, t0 * P:(t0 + CH) * P])
            nc.sync.dma_start(ge[:], io["efT"][:, t0 * P:(t0 + CH) * P])

            psKP = pp.tile([P, 2 * CHE], F32, tag="psKP")   # K cols 0:512, P cols 512:1024
            psQ = pp.tile([P, CHE], F32, tag="psQ")
            psV = pp.tile([P, CHE], F32, tag="psV")

            for j in range(CH):
                sl = slice(j * P, (j + 1) * P)
                nc.tensor.matmul(psKP[:, j * P:(j + 1) * P], lhsT=gs[:, sl],
                                 rhs=wf["Wk"][:], start=(j == 0), stop=False)
                nc.tensor.matmul(psV[:, sl], lhsT=gs[:, sl], rhs=wf["Wv"][:],
                                 start=(j == 0), stop=(j == CH - 1))
                nc.tensor.matmul(psQ[:, sl], lhsT=gd[:, sl], rhs=wf["Wq"][:],
                                 start=(j == 0), stop=False)
                nc.tensor.matmul(psKP[:, CHE + j * P:CHE + (j + 1) * P], lhsT=ge[:, sl],
                                 rhs=wf["We"][:], start=(j == 0), stop=False)
            nc.tensor.matmul(psKP[:, :CHE], lhsT=ones_row[:],
                             rhs=brow["bk"][:], start=False, stop=True)
            nc.tensor.matmul(psQ[:], lhsT=ones_row[:],
                             rhs=brow["bq"][:], start=False, stop=True)
            nc.tensor.matmul(psKP[:, CHE:], lhsT=ones_row[:],
                             rhs=brow["be"][:], start=False, stop=True)
            kp = mp.tile([P, 2 * CHE], BF, tag="kp")
            nc.scalar.copy(kp[:], psKP[:])
            tmp = mp.tile([P, CHE], BF, tag="tmp")
            nc.vector.tensor_tensor(out=tmp[:], in0=kp[:, :CHE], in1=psQ[:], op=OP.mult)
            u = mp.tile([P, CHE], BF, tag="u")
            nc.gpsimd.tensor_tensor(out=u[:], in0=tmp[:], in1=kp[:, CHE:], op=OP.mult)
            spre = mp.tile([P, CH * H], F32, tag="spre")
            nc.vector.tensor_reduce(
                out=spre[:],
                in_=u[:].rearrange("p (t h d) -> p (t h) d", d=DH, h=H),
                axis=AX.X, op=OP.add)
            sexp = mp.tile([P, CH * H], F32, tag="sexp")
            nc.scalar.activation(sexp[:], spre[:], AF.Exp, scale=DH_SCALE)

            sv = mp.tile([P, CH * (D + H)], BF, tag="sv")
            sv3 = sv[:].rearrange("p (t c) -> p t c", c=D + H)
            nc.vector.tensor_scalar(
                out=sv3[:, :, D:D + H],
                in0=sexp[:].rearrange("p (t h) -> p t h", h=H),
                scalar1=EXP5, scalar2=EXPM5, op0=OP.min, op1=OP.max)
            nc.vector.tensor_tensor(
                out=sv3[:, :, :D].rearrange("p t (h d) -> p t h d", d=DH),
                in0=psV[:].rearrange("p (t h d) -> p t h d", h=H, d=DH),
                in1=sv3[:, :, D:D + H].to_broadcast([P, CH, H, DH]),
                op=OP.mult)

            M4 = mp.tile([P, CHE], BF, tag="M4")
            for j in range(CH):
                gt = t0 + j
                nc.gpsimd.tensor_scalar(
                    out=M4[:, j * P:(j + 1) * P], in0=iota_f[:],
                    scalar1=dstw[:, gt:gt + 1], scalar2=None, op0=OP.is_equal)

            for j in range(CH):
                gt = t0 + j
                w = t2w[gt]
                first = (gt == Woff[w])
                last = (gt == Woff[w] + Tw[w] - 1)
                if first:
                    scat_tiles[w] = sp.tile([P, D + H], F32, tag="scat", name=f"scat_{w}")
                nc.tensor.matmul(
                    scat_tiles[w][:], lhsT=M4[:, j * P:(j + 1) * P],
                    rhs=sv3[:, j, :],
                    start=first, stop=last)
                if last:
                    window_evac(w, scat_tiles.pop(w)[:])

    with tc.tile_critical():
        nc.all_engine_barrier()

    # ---- phase 4: node pipeline ------------------------------------------
    hres = res.tile([P, NPAD], F32)
    nchn = (NPAD + CHE - 1) // CHE
    b2st6 = stat.tile([P, nchn * 6], F32)

    with (
        tc.tile_pool(name="nodes", bufs=2) as np_pool,
        tc.tile_pool(name="node_ps", bufs=2, space="PSUM") as nps,
    ):
        for i in range(nchn):
            lo = i * CHE
            hi = min(lo + CHE, NPAD)
            n = hi - lo
            ps = nps.tile([P, CHE], F32, tag="psO")
            nc.tensor.matmul(ps[:, :n], lhsT=Ow_b[:],
                             rhs=hT[:, lo:hi], start=True, stop=True)
            ob = np_pool.tile([P, CHE], F32, tag="ob")
            nc.scalar.activation(ob[:, :n], ps[:, :n], AF.Identity,
                                 bias=vec_sb["O_b"][:, :1])
            nc.vector.tensor_tensor(out=hres[:, lo:hi], in0=ob[:, :n],
                                    in1=nfT[:, lo:hi], op=OP.add)
        if NPAD > NPC:
            nc.vector.memset(hres[:, NPC:NPAD], 0.0)
        for i in range(nchn):
            lo = i * CHE; hi = min(lo + CHE, NPAD)
            nc.vector.bn_stats(b2st6[:, i * 6:(i + 1) * 6], hres[:, lo:hi])
        b2sx, b2sxx = combine6(b2st6, nchn, "2")
        stats2 = stat.tile([P, 2], F32)
        nc.vector.tensor_copy(stats2[:, 0:1], b2sx[:])
        nc.vector.tensor_copy(stats2[:, 1:2], b2sxx[:])
        cc2_in = dram.tile([P, 2], F32)
        cc2_out = dram.tile([P, 2], F32)
        nc.sync.dma_start(cc2_in[:], stats2[:])
        nc.gpsimd.collective_compute(
            "AllReduce", OP.add, replica_groups=cc_group,
            ins=[cc2_in.opt()], outs=[cc2_out.opt()])
        gstats2 = stat.tile([P, 2], F32)
        nc.sync.dma_start(gstats2[:], cc2_out[:])

        scl2, cb2 = bn_coeffs(gstats2[:, 0:1], gstats2[:, 1:2], N,
                              vec_sb["g2"][:], vec_sb["b2"][:], "2")

        for i in range(nchn):
            lo = i * CHE; hi = min(lo + CHE, NPAD); n = hi - lo
            bn = np_pool.tile([P, CHE], BF, tag="bn2")
            nc.vector.tensor_scalar(out=bn[:, :n], in0=hres[:, lo:hi],
                                    scalar1=scl2[:, :1], scalar2=cb2[:, :1],
                                    op0=OP.mult, op1=OP.add)
            ps1 = nps.tile([P, CHE], F32, tag="ps1")
            ps2 = nps.tile([P, CHE], F32, tag="ps2")
            nc.tensor.matmul(ps1[:, :n], lhsT=W1_b[:, :P],
                             rhs=bn[:, :n], start=True, stop=True)
            nc.tensor.matmul(ps2[:, :n], lhsT=W1_b[:, P:],
                             rhs=bn[:, :n], start=True, stop=True)
            sg1 = np_pool.tile([P, CHE], F32, tag="sg1")
            sg2 = np_pool.tile([P, CHE], F32, tag="sg2")
            nc.scalar.activation(sg1[:, :n], ps1[:, :n], AF.Sigmoid)
            nc.scalar.activation(sg2[:, :n], ps2[:, :n], AF.Sigmoid)
            sl1 = np_pool.tile([P, CHE], BF, tag="sl1")
            sl2 = np_pool.tile([P, CHE], BF, tag="sl2")
            nc.vector.tensor_tensor(out=sl1[:, :n], in0=sg1[:, :n], in1=ps1[:, :n], op=OP.mult)
            nc.vector.tensor_tensor(out=sl2[:, :n], in0=sg2[:, :n], in1=ps2[:, :n], op=OP.mult)
            psM = nps.tile([P, CHE], F32, tag="psM")
            nc.tensor.matmul(psM[:, :n], lhsT=W2_b[:, :P],
                             rhs=sl1[:, :n], start=True, stop=False)
            nc.tensor.matmul(psM[:, :n], lhsT=W2_b[:, P:],
                             rhs=sl2[:, :n], start=False, stop=True)
            outc = np_pool.tile([P, CHE], F32, tag="outc")
            nc.vector.tensor_tensor(out=outc[:, :n], in0=psM[:, :n],
                                    in1=hres[:, lo:hi], op=OP.add)
            nc.sync.dma_start(io["outT"][:, lo:hi], outc[:, :n])


# ----------------------------------------------------------------------------
# Entry point
# ----------------------------------------------------------------------------

def declare_io(nc, cfg, meta):
    NPAD, TC, EPC = meta["NPAD"], meta["TC"], meta["EPC"]
    io = {}
    io["gsT"] = nc.declare_dram_parameter("gsT", [P, EPC], BF, isOutput=False)
    io["gdT"] = nc.declare_dram_parameter("gdT", [P, EPC], BF, isOutput=False)
    io["efT"] = nc.declare_dram_parameter("efT", [P, EPC], BF, isOutput=False)
    io["dstw"] = nc.declare_dram_parameter("dstw", [P, TC], F32, isOutput=False)
    io["nfT"] = nc.declare_dram_parameter("nfT", [P, NPAD], F32, isOutput=False)
    for w, sh in (("Wq", [P, P]), ("Wk", [P, P]), ("Wv", [P, P]), ("We", [P, P]),
                  ("O_w", [P, P]), ("W1", [P, 2 * P]), ("W2", [2 * P, P])):
        io[w] = nc.declare_dram_parameter(w, sh, F32, isOutput=False)
    for v in ("O_b", "g1n", "b1n", "g1e", "b1e", "g2", "b2"):
        io[v] = nc.declare_dram_parameter(v, [P, 1], F32, isOutput=False)
    io["outT"] = nc.declare_dram_parameter("outT", [P, NPAD], F32, isOutput=True)
    return io


def build_program(cfg, meta):
    nc = bacc.Bacc("TRN2", target_bir_lowering=False, debug=False,
                   num_devices=cfg["NCORES"])
    io = declare_io(nc, cfg, meta)
    with ExitStack() as ctx:
        tc = ctx.enter_context(tile.TileContext(nc, num_cores=cfg["NCORES"]))
        build_graph(ctx, tc, io, cfg, meta)
    nc.compile()
    return nc


def assemble_output(results, cfg, meta):
    NPC = meta["NPC"]
    out = np.empty((cfg["N"], cfg["D"]), np.float32)
    for c in range(cfg["NCORES"]):
        out[c * NPC:(c + 1) * NPC] = np.asarray(results[c]["outT"], np.float32).T[:NPC]
    return out


def kernel(**inputs):
    from concourse.bass_utils import run_bass_kernel_spmd

    cfg = default_cfg()
    in_maps, meta = host_prep(inputs, cfg)
    nc = build_program(cfg, meta)
    res = run_bass_kernel_spmd(nc, in_maps, core_ids=list(range(cfg["NCORES"])))
    return assemble_output(res.results, cfg, meta)
